# revision 15
# baseline (speedup 1.0000x reference)
"""HEALPix padding (p=2) kernel for Trainium2 (Bass/Tile).

Input : data (96, 256, 64, 64) f32 = (B*12 faces, C, H, W), B=8, plus scalar p=2.
Output: (96, 256, 68, 68) f32.

Sharding: data-parallel over the batch dim. Each of the 8 NeuronCores gets one
group of 12 HEALPix faces (12, 256, 64, 64) so every cross-face halo gather is
core-local.

Per-core plan (per 128-channel chunk, channels on SBUF partitions):
  - Face tiles stream through SBUF (contiguous 2MB loads). On arrival the
    tile's 4 edge strips (first/last-2 rows and cols) are extracted on-chip
    and the interior is copied into a 64x68 "mid" block.
  - Each padded face is stored as three row-contiguous pieces, so every DMA
    moves >=512B-contiguous runs at full modeled bandwidth:
      top strip  y[g, :, 0:2, :]  (2x68)
      mid block  y[g, :, 2:66, :] (64x68 = left halo cols | interior | right)
      bottom     y[g, :, 66:68, :]
    A piece's halo cells come only from the staged edge strips, so each piece
    becomes storable as soon as its 2-3 source faces have loaded.
  - All DMAs issue in program order from the SP sequencer. The Tile
    framework's 8 DMAHW completion lanes give each DMA a slot-reuse wait on
    its lane's 8-back predecessor — long satisfied by issue time, so
    back-to-back transfers never pay a sem-prop + descriptor-gen bubble.
    walrus only accepts one sync wait per instruction, so a post-pass moves
    excess waits onto no-op EventSemaphore carriers inserted just before the
    owning instruction (sound on in-order sequencers; see _build_nc).
The face load order keeps at most ~4 mid blocks live while letting stores
interleave with loads from the second load onward; piece emission is paced
(CAP per load step) so chunk-0's tail stores interleave with chunk-1's first
loads. The first load is hoisted above the startup barrier (see _build_nc).
TimelineSim: 300593 ns = first-DMA latency (1575) + DMA-device busy floor
(297624 = 107.1MB at 360B/ns) + drain tail (1394), with zero idle between
the first load and the last store.
"""

import numpy as np

_FACES = 12
_PAD = 2

# Load order: stores become ready early and steadily; peak live mids stays
# small and the final face unlocks only cheap pieces, so the DMA engines
# run gap-free from the first load to the last store.
_ORDER = [9, 8, 10, 4, 5, 3, 0, 11, 2, 6, 1, 7]


def _face_tables(g):
    """Neighbor faces for face g, keyed by role."""
    if g < 4:  # _pn
        i = g
        return dict(kind="pn", t=(i + 1) % 4, tl=(i + 2) % 4, l=(i + 3) % 4,
                    bl=(i + 3) % 4, b=4 + i, br=8 + i, r=4 + (i + 1) % 4,
                    tr=(i + 1) % 4)
    if g < 8:  # _pe
        i = g - 4
        return dict(kind="pe", t=i, l=(i + 3) % 4, bl=4 + (i + 3) % 4,
                    b=8 + (i + 3) % 4, r=8 + i, tr=4 + (i + 1) % 4)
    i = g - 8  # _ps
    return dict(kind="ps", t=4 + (i + 1) % 4, tl=i, l=4 + i,
                bl=8 + (i + 3) % 4, b=8 + (i + 3) % 4, r=8 + (i + 1) % 4,
                tr=8 + (i + 1) % 4, br=8 + (i + 2) % 4)


def _piece_deps(g):
    """Faces whose staged edges each piece of face g reads."""
    tb = _face_tables(g)
    if tb["kind"] == "pn":
        return dict(mid={g, tb["l"], tb["r"]},
                    top={tb["t"], tb["tl"]},
                    bot={tb["b"], tb["bl"], tb["br"]})
    if tb["kind"] == "pe":
        return dict(mid={g, tb["l"], tb["r"]},
                    top={tb["t"], tb["l"], tb["tr"]},
                    bot={tb["b"], tb["bl"], tb["r"]})
    return dict(mid={g, tb["l"], tb["r"]},
                top={tb["t"], tb["tl"], tb["tr"]},
                bot={tb["b"], tb["br"], tb["bl"]})


def _build_nc(C=256, H=64, PCHUNK=128):
    import concourse.bass as bass
    import concourse.mybir as mybir
    from concourse.tile import TileContext

    f32 = mybir.dt.float32
    W = H
    OH = H + 2 * _PAD
    P = PCHUNK
    nc = bass.Bass()
    x = nc.dram_tensor("data", (_FACES, C, H, W), f32, kind="ExternalInput")
    y = nc.dram_tensor("out", (_FACES, C, OH, OH), f32, kind="ExternalOutput")
    V = nc.vector

    with TileContext(nc) as tc:
        with (
            tc.tile_pool(name="tiles", bufs=3) as tpool,
            tc.tile_pool(name="mids", bufs=4) as mpool,
            tc.tile_pool(name="rows", bufs=4) as rpool,
            tc.tile_pool(name="cols", bufs=4) as cpool,
            tc.tile_pool(name="strips", bufs=24) as spool,
        ):
            def make_chunk(c0):
                cs = slice(c0, c0 + P)
                toprows = rpool.tile([P, _FACES, 2, W], f32,
                                     name=f"toprows_{c0}", tag="rows")
                botrows = rpool.tile([P, _FACES, 2, W], f32,
                                     name=f"botrows_{c0}", tag="rows")
                colL = cpool.tile([P, _FACES, H, 2], f32,
                                  name=f"colL_{c0}", tag="cols")
                colR = cpool.tile([P, _FACES, H, 2], f32,
                                  name=f"colR_{c0}", tag="cols")
                tr_f = toprows.rearrange("p f r w -> p (f r w)")
                br_f = botrows.rearrange("p f r w -> p (f r w)")

                mids = {}
                loaded, emitted = set(), set()

                def emit_mid(g):
                    tb = _face_tables(g)
                    mid = mids[g]
                    if tb["kind"] == "pn":
                        for j in range(2):
                            V.tensor_copy(mid[:, :, j], toprows[:, tb["l"], 1 - j, :])
                        V.tensor_copy(mid[:, :, W + 2:W + 4], colL[:, tb["r"]])
                    elif tb["kind"] == "pe":
                        V.tensor_copy(mid[:, :, 0:2], colR[:, tb["l"]])
                        V.tensor_copy(mid[:, :, W + 2:W + 4], colL[:, tb["r"]])
                    else:
                        V.tensor_copy(mid[:, :, 0:2], colR[:, tb["l"]])
                        for j in range(2):
                            V.tensor_copy(mid[:, :, W + 2 + j], botrows[:, tb["r"], 1 - j, :])
                    nc.sync.dma_start(
                        out=y[g, cs, 2:2 + H, :].rearrange("c a b -> c (a b)"),
                        in_=mid.rearrange("p a b -> p (a b)"))

                def emit_top(g):
                    tb = _face_tables(g)
                    st = spool.tile([P, 2, OH], f32, name=f"top_{c0}_{g}",
                                    tag="strip")
                    st_f = st.rearrange("p a b -> p (a b)")
                    if tb["kind"] == "pn":
                        for r_ in range(2):
                            V.tensor_copy(st[:, r_, 2:2 + W], colL[:, tb["t"], :, 1 - r_])
                        for i_ in range(2):
                            for j_ in range(2):
                                V.tensor_copy(st[:, i_:i_ + 1, j_:j_ + 1],
                                              toprows[:, tb["tl"], 1 - i_:2 - i_, 1 - j_:2 - j_])
                        V.tensor_copy(st[:, 0:2, W + 2:W + 4], botrows[:, tb["tr"], :, 0:2])
                    elif tb["kind"] == "pe":
                        t, l = tb["t"], tb["l"]
                        V.tensor_copy(st[:, 0:2, 2:2 + W], botrows[:, t, :, :])
                        V.tensor_copy(st[:, 0:1, 1:2], botrows[:, t, 0:1, 0:1])
                        V.tensor_copy(st[:, 1:2, 0:1], toprows[:, l, 0:1, W - 2:W - 1])
                        d = st_f[:, 0:OH + 2:OH + 1]
                        V.tensor_add(d, br_f[:, t * 2 * W:t * 2 * W + W + 1:W],
                                     tr_f[:, l * 2 * W + W - 2:l * 2 * W + W])
                        V.tensor_scalar_mul(d, d, 0.5)
                        V.tensor_copy(st[:, 0:2, W + 2:W + 4], botrows[:, tb["tr"], :, 0:2])
                    else:
                        V.tensor_copy(st[:, 0:2, 2:2 + W], botrows[:, tb["t"], :, :])
                        V.tensor_copy(st[:, 0:2, 0:2], botrows[:, tb["tl"], :, W - 2:W])
                        V.tensor_copy(st[:, 0:2, W + 2:W + 4], botrows[:, tb["tr"], :, 0:2])
                    nc.sync.dma_start(
                        out=y[g, cs, 0:2, :].rearrange("c a b -> c (a b)"),
                        in_=st_f)

                def emit_bot(g):
                    tb = _face_tables(g)
                    sb = spool.tile([P, 2, OH], f32, name=f"bot_{c0}_{g}",
                                    tag="strip")
                    sb_f = sb.rearrange("p a b -> p (a b)")
                    if tb["kind"] == "pn":
                        V.tensor_copy(sb[:, 0:2, 2:2 + W], toprows[:, tb["b"], :, :])
                        V.tensor_copy(sb[:, 0:2, 0:2], toprows[:, tb["bl"], :, W - 2:W])
                        V.tensor_copy(sb[:, 0:2, W + 2:W + 4], toprows[:, tb["br"], :, 0:2])
                    elif tb["kind"] == "pe":
                        b, r = tb["b"], tb["r"]
                        V.tensor_copy(sb[:, 0:2, 2:2 + W], toprows[:, b, :, :])
                        V.tensor_copy(sb[:, 0:2, 0:2], toprows[:, tb["bl"], :, W - 2:W])
                        V.tensor_copy(sb[:, 0:1, W + 3:W + 4], botrows[:, r, 1:2, 1:2])
                        V.tensor_copy(sb[:, 1:2, W + 2:W + 3], toprows[:, b, 1:2, W - 1:W])
                        d = sb_f[:, W + 2:W + 2 + OH + 2:OH + 1]
                        V.tensor_add(d, tr_f[:, b * 2 * W + W - 1:b * 2 * W + 2 * W:W],
                                     br_f[:, r * 2 * W + W:r * 2 * W + W + 2])
                        V.tensor_scalar_mul(d, d, 0.5)
                    else:
                        b, br = tb["b"], tb["br"]
                        for r_ in range(2):
                            V.tensor_copy(sb[:, r_, 2:2 + W], colR[:, b, :, 1 - r_])
                        V.tensor_copy(sb[:, 0:2, 0:2], toprows[:, tb["bl"], :, W - 2:W])
                        for i_ in range(2):
                            for j_ in range(2):
                                V.tensor_copy(sb[:, i_:i_ + 1, W + 2 + j_:W + 3 + j_],
                                              botrows[:, br, 1 - i_:2 - i_, W - 1 - j_:W - j_])
                    nc.sync.dma_start(
                        out=y[g, cs, H + 2:H + 4, :].rearrange("c a b -> c (a b)"),
                        in_=sb_f)

                emitters = dict(mid=emit_mid, top=emit_top, bot=emit_bot)

                def load(f, pending):
                    tile = tpool.tile([P, H, W], f32,
                                      name=f"tile_{c0}_{f}", tag="tile")
                    nc.sync.dma_start(
                        out=tile.rearrange("p a b -> p (a b)"),
                        in_=x[f, cs].rearrange("c a b -> c (a b)"))
                    V.tensor_copy(toprows[:, f], tile[:, 0:2, :])
                    V.tensor_copy(botrows[:, f], tile[:, H - 2:H, :])
                    V.tensor_copy(colL[:, f], tile[:, :, 0:2])
                    V.tensor_copy(colR[:, f], tile[:, :, W - 2:W])
                    mid = mpool.tile([P, H, OH], f32,
                                     name=f"mid_{c0}_{f}", tag="mid")
                    V.tensor_copy(mid[:, :, 2:2 + W], tile[:])
                    mids[f] = mid
                    loaded.add(f)
                    for g in _ORDER:
                        for piece, deps in _piece_deps(g).items():
                            if (g, piece) in emitted or not deps <= loaded:
                                continue
                            emitted.add((g, piece))
                            pending.append((emitters[piece], piece, g))

                return load, emitted

            # Flat schedule over both 128-channel chunks: pieces are emitted
            # through a paced queue so chunk-0's tail stores interleave with
            # chunk-1's first loads instead of starving the DMA engines on
            # back-to-back descriptor generation of tiny strip stores. Mid
            # stores drain before strips: a 6.2us mid transfer hides the
            # HWDGE generation time of the following strips.
            chunks = [make_chunk(c0) for c0 in range(0, C, PCHUNK)]
            pending = []
            CAP = 4
            for s in range(len(chunks) * _FACES):
                chunks[s // _FACES][0](_ORDER[s % _FACES], pending)
                pending.sort(key=lambda e: e[1] != "mid")
                for _ in range(min(CAP, len(pending))):
                    fn, piece, g = pending.pop(0)
                    fn(g)
            tail_mids = [e for e in pending if e[1] == "mid"]
            tail_rest = [e for e in pending if e[1] != "mid"]
            for fn, piece, g in tail_mids[:1] + tail_rest + tail_mids[1:]:
                fn(g)
            for _, emitted in chunks:
                assert len(emitted) == 3 * _FACES

    # walrus's CoreV2 codegen accepts a single sync-wait slot per instruction
    # ("Too many sync wait commands" in setupSyncWait), but the Tile framework
    # legitimately gives some instructions several waits: DMAs get a compute
    # wait plus their DMAHW-lane slot-reuse wait (which keeps each lane's
    # completion updates ordered — required for the shared counting-sem
    # protocol on hardware, where DMA engines finish out of order), and the
    # kernel-tail Drains join multiple sems. Splitting is sound on in-order
    # sequencers: excess waits move to no-op InstEventSemaphore carriers
    # inserted immediately before the instruction on the same engine, so the
    # original instruction still cannot issue before every wait is satisfied.
    # DMAs keep the wait on the DMAHW lane they update so each lane's +16
    # completion updates stay ordered with its predecessor's.
    import concourse.mybir as mybir
    for blk in nc.m.functions[0].blocks:
        out_insts = []
        for inst in blk.instructions:
            si = inst.sync_info
            if si is None or len(si.on_wait) <= 1:
                out_insts.append(inst)
                continue
            waits = list(si.on_wait)
            own_lanes = {u.ant_name for u in si.on_update}
            keep_idx = next((i for i, w in enumerate(waits)
                             if w.ant_name in own_lanes), len(waits) - 1)
            keep = waits.pop(keep_idx)
            for w in waits:
                carrier = mybir.InstEventSemaphore(
                    name=f"I-{nc.next_id()}", engine=inst.engine)
                carrier.sync_info = mybir.SyncInfo(on_wait=[w], on_update=[])
                nc.inst_map[carrier.name] = carrier
                out_insts.append(carrier)
            si.on_wait = [keep]
            inst.sync_info = si
            out_insts.append(inst)
        blk.instructions = out_insts

    for blk in nc.m.functions[0].blocks:
        for inst in blk.instructions:
            si = inst.sync_info
            assert si is None or len(si.on_wait) <= 1, inst.concise()

    # The first load has no waits, but sits after the 5-engine startup
    # barrier (~700ns) purely by block position. Hoist it into the preamble
    # block directly after SP's entry Drain: the Drain's sem/DMA-state reset
    # still precedes it on the in-order SP stream, and its completion-sem
    # update lands only after the ~6us transfer — microseconds after every
    # engine's init Drain — so the barrier provides it no ordering it needs.
    # This starts the first transfer ~760ns earlier.
    b0 = nc.m.functions[0].blocks[0]
    b1 = nc.m.functions[0].blocks[1]
    first = b1.instructions[0]
    assert isinstance(first, mybir.InstDMACopy), first.concise()
    assert first.engine == mybir.EngineType.SP, first.concise()
    assert first.sync_info is None or not first.sync_info.on_wait, first.concise()
    sp_drain = [i for i, x in enumerate(b0.instructions)
                if x.engine == mybir.EngineType.SP and isinstance(x, mybir.InstDrain)]
    assert len(sp_drain) == 1, sp_drain
    b1.instructions = b1.instructions[1:]
    b0.instructions = (b0.instructions[:sp_drain[0] + 1] + [first]
                       + b0.instructions[sp_drain[0] + 1:])

    nc.finalize()
    return nc


_NC_CACHE = {}


def _get_nc():
    if "nc" not in _NC_CACHE:
        _NC_CACHE["nc"] = _build_nc()
    return _NC_CACHE["nc"]


def _run(data, **kwargs):
    from concourse import bass_utils

    data = np.ascontiguousarray(np.asarray(data, dtype=np.float32))
    n_cores = 8
    group = data.shape[0] // n_cores
    assert group == _FACES
    nc = _get_nc()
    in_maps = [{"data": data[g * group:(g + 1) * group]} for g in range(n_cores)]
    return bass_utils.run_bass_kernel_spmd(
        nc, in_maps, core_ids=list(range(n_cores)), **kwargs)


def kernel(data, p):
    assert int(p) == _PAD
    res = _run(data)
    return np.concatenate([r["out"] for r in res.results], axis=0)


# revision 16
# speedup vs baseline: 1.0008x; 1.0008x over previous
"""HEALPix padding (p=2) kernel for Trainium2 (Bass/Tile).

Input : data (96, 256, 64, 64) f32 = (B*12 faces, C, H, W), B=8, plus scalar p=2.
Output: (96, 256, 68, 68) f32.

Sharding: data-parallel over the batch dim. Each of the 8 NeuronCores gets one
group of 12 HEALPix faces (12, 256, 64, 64) so every cross-face halo gather is
core-local.

Per-core plan (per 128-channel chunk, channels on SBUF partitions):
  - Face tiles stream through SBUF (contiguous 2MB loads). On arrival the
    tile's 4 edge strips (first/last-2 rows and cols) are extracted on-chip
    and the interior is copied into a 64x68 "mid" block.
  - Each padded face is stored as three row-contiguous pieces, so every DMA
    moves >=512B-contiguous runs at full modeled bandwidth:
      top strip  y[g, :, 0:2, :]  (2x68)
      mid block  y[g, :, 2:66, :] (64x68 = left halo cols | interior | right)
      bottom     y[g, :, 66:68, :]
    A piece's halo cells come only from the staged edge strips, so each piece
    becomes storable as soon as its 2-3 source faces have loaded.
  - All DMAs issue in program order from the SP sequencer. The Tile
    framework's 8 DMAHW completion lanes give each DMA a slot-reuse wait on
    its lane's 8-back predecessor — long satisfied by issue time, so
    back-to-back transfers never pay a sem-prop + descriptor-gen bubble.
    walrus only accepts one sync wait per instruction, so a post-pass moves
    excess waits onto no-op EventSemaphore carriers inserted just before the
    owning instruction (sound on in-order sequencers; see _build_nc).
The face load order keeps at most ~4 mid blocks live while letting stores
interleave with loads from the second load onward; piece emission is paced
(CAP per load step) so chunk-0's tail stores interleave with chunk-1's first
loads. The first load is hoisted above the startup barrier (see _build_nc).
TimelineSim: 300593 ns = first-DMA latency (1575) + DMA-device busy floor
(297624 = 107.1MB at 360B/ns) + drain tail (1394), with zero idle between
the first load and the last store.
"""

import numpy as np

_FACES = 12
_PAD = 2

# Load order: stores become ready early and steadily; peak live mids stays
# small and the final face unlocks only cheap pieces, so the DMA engines
# run gap-free from the first load to the last store.
_ORDER = [9, 8, 10, 4, 5, 3, 0, 11, 2, 6, 1, 7]


def _face_tables(g):
    """Neighbor faces for face g, keyed by role."""
    if g < 4:  # _pn
        i = g
        return dict(kind="pn", t=(i + 1) % 4, tl=(i + 2) % 4, l=(i + 3) % 4,
                    bl=(i + 3) % 4, b=4 + i, br=8 + i, r=4 + (i + 1) % 4,
                    tr=(i + 1) % 4)
    if g < 8:  # _pe
        i = g - 4
        return dict(kind="pe", t=i, l=(i + 3) % 4, bl=4 + (i + 3) % 4,
                    b=8 + (i + 3) % 4, r=8 + i, tr=4 + (i + 1) % 4)
    i = g - 8  # _ps
    return dict(kind="ps", t=4 + (i + 1) % 4, tl=i, l=4 + i,
                bl=8 + (i + 3) % 4, b=8 + (i + 3) % 4, r=8 + (i + 1) % 4,
                tr=8 + (i + 1) % 4, br=8 + (i + 2) % 4)


def _piece_deps(g):
    """Faces whose staged edges each piece of face g reads."""
    tb = _face_tables(g)
    if tb["kind"] == "pn":
        return dict(mid={g, tb["l"], tb["r"]},
                    top={tb["t"], tb["tl"]},
                    bot={tb["b"], tb["bl"], tb["br"]})
    if tb["kind"] == "pe":
        return dict(mid={g, tb["l"], tb["r"]},
                    top={tb["t"], tb["l"], tb["tr"]},
                    bot={tb["b"], tb["bl"], tb["r"]})
    return dict(mid={g, tb["l"], tb["r"]},
                top={tb["t"], tb["tl"], tb["tr"]},
                bot={tb["b"], tb["br"], tb["bl"]})


def _build_nc(C=256, H=64, PCHUNK=128):
    import concourse.bass as bass
    import concourse.mybir as mybir
    from concourse.tile import TileContext

    f32 = mybir.dt.float32
    W = H
    OH = H + 2 * _PAD
    P = PCHUNK
    nc = bass.Bass()
    x = nc.dram_tensor("data", (_FACES, C, H, W), f32, kind="ExternalInput")
    y = nc.dram_tensor("out", (_FACES, C, OH, OH), f32, kind="ExternalOutput")
    V = nc.vector

    with TileContext(nc) as tc:
        with (
            tc.tile_pool(name="tiles", bufs=3) as tpool,
            tc.tile_pool(name="mids", bufs=4) as mpool,
            tc.tile_pool(name="rows", bufs=4) as rpool,
            tc.tile_pool(name="cols", bufs=4) as cpool,
            tc.tile_pool(name="strips", bufs=24) as spool,
        ):
            def make_chunk(c0):
                cs = slice(c0, c0 + P)
                toprows = rpool.tile([P, _FACES, 2, W], f32,
                                     name=f"toprows_{c0}", tag="rows")
                botrows = rpool.tile([P, _FACES, 2, W], f32,
                                     name=f"botrows_{c0}", tag="rows")
                colL = cpool.tile([P, _FACES, H, 2], f32,
                                  name=f"colL_{c0}", tag="cols")
                colR = cpool.tile([P, _FACES, H, 2], f32,
                                  name=f"colR_{c0}", tag="cols")
                tr_f = toprows.rearrange("p f r w -> p (f r w)")
                br_f = botrows.rearrange("p f r w -> p (f r w)")

                mids = {}
                loaded, emitted = set(), set()

                def emit_mid(g):
                    tb = _face_tables(g)
                    mid = mids[g]
                    if tb["kind"] == "pn":
                        for j in range(2):
                            V.tensor_copy(mid[:, :, j], toprows[:, tb["l"], 1 - j, :])
                        V.tensor_copy(mid[:, :, W + 2:W + 4], colL[:, tb["r"]])
                    elif tb["kind"] == "pe":
                        V.tensor_copy(mid[:, :, 0:2], colR[:, tb["l"]])
                        V.tensor_copy(mid[:, :, W + 2:W + 4], colL[:, tb["r"]])
                    else:
                        V.tensor_copy(mid[:, :, 0:2], colR[:, tb["l"]])
                        for j in range(2):
                            V.tensor_copy(mid[:, :, W + 2 + j], botrows[:, tb["r"], 1 - j, :])
                    nc.sync.dma_start(
                        out=y[g, cs, 2:2 + H, :].rearrange("c a b -> c (a b)"),
                        in_=mid.rearrange("p a b -> p (a b)"))

                def emit_top(g):
                    tb = _face_tables(g)
                    st = spool.tile([P, 2, OH], f32, name=f"top_{c0}_{g}",
                                    tag="strip")
                    st_f = st.rearrange("p a b -> p (a b)")
                    if tb["kind"] == "pn":
                        for r_ in range(2):
                            V.tensor_copy(st[:, r_, 2:2 + W], colL[:, tb["t"], :, 1 - r_])
                        for i_ in range(2):
                            for j_ in range(2):
                                V.tensor_copy(st[:, i_:i_ + 1, j_:j_ + 1],
                                              toprows[:, tb["tl"], 1 - i_:2 - i_, 1 - j_:2 - j_])
                        V.tensor_copy(st[:, 0:2, W + 2:W + 4], botrows[:, tb["tr"], :, 0:2])
                    elif tb["kind"] == "pe":
                        t, l = tb["t"], tb["l"]
                        V.tensor_copy(st[:, 0:2, 2:2 + W], botrows[:, t, :, :])
                        V.tensor_copy(st[:, 0:1, 1:2], botrows[:, t, 0:1, 0:1])
                        V.tensor_copy(st[:, 1:2, 0:1], toprows[:, l, 0:1, W - 2:W - 1])
                        d = st_f[:, 0:OH + 2:OH + 1]
                        V.tensor_add(d, br_f[:, t * 2 * W:t * 2 * W + W + 1:W],
                                     tr_f[:, l * 2 * W + W - 2:l * 2 * W + W])
                        V.tensor_scalar_mul(d, d, 0.5)
                        V.tensor_copy(st[:, 0:2, W + 2:W + 4], botrows[:, tb["tr"], :, 0:2])
                    else:
                        V.tensor_copy(st[:, 0:2, 2:2 + W], botrows[:, tb["t"], :, :])
                        V.tensor_copy(st[:, 0:2, 0:2], botrows[:, tb["tl"], :, W - 2:W])
                        V.tensor_copy(st[:, 0:2, W + 2:W + 4], botrows[:, tb["tr"], :, 0:2])
                    nc.sync.dma_start(
                        out=y[g, cs, 0:2, :].rearrange("c a b -> c (a b)"),
                        in_=st_f)

                def emit_bot(g):
                    tb = _face_tables(g)
                    sb = spool.tile([P, 2, OH], f32, name=f"bot_{c0}_{g}",
                                    tag="strip")
                    sb_f = sb.rearrange("p a b -> p (a b)")
                    if tb["kind"] == "pn":
                        V.tensor_copy(sb[:, 0:2, 2:2 + W], toprows[:, tb["b"], :, :])
                        V.tensor_copy(sb[:, 0:2, 0:2], toprows[:, tb["bl"], :, W - 2:W])
                        V.tensor_copy(sb[:, 0:2, W + 2:W + 4], toprows[:, tb["br"], :, 0:2])
                    elif tb["kind"] == "pe":
                        b, r = tb["b"], tb["r"]
                        V.tensor_copy(sb[:, 0:2, 2:2 + W], toprows[:, b, :, :])
                        V.tensor_copy(sb[:, 0:2, 0:2], toprows[:, tb["bl"], :, W - 2:W])
                        V.tensor_copy(sb[:, 0:1, W + 3:W + 4], botrows[:, r, 1:2, 1:2])
                        V.tensor_copy(sb[:, 1:2, W + 2:W + 3], toprows[:, b, 1:2, W - 1:W])
                        d = sb_f[:, W + 2:W + 2 + OH + 2:OH + 1]
                        V.tensor_add(d, tr_f[:, b * 2 * W + W - 1:b * 2 * W + 2 * W:W],
                                     br_f[:, r * 2 * W + W:r * 2 * W + W + 2])
                        V.tensor_scalar_mul(d, d, 0.5)
                    else:
                        b, br = tb["b"], tb["br"]
                        for r_ in range(2):
                            V.tensor_copy(sb[:, r_, 2:2 + W], colR[:, b, :, 1 - r_])
                        V.tensor_copy(sb[:, 0:2, 0:2], toprows[:, tb["bl"], :, W - 2:W])
                        for i_ in range(2):
                            for j_ in range(2):
                                V.tensor_copy(sb[:, i_:i_ + 1, W + 2 + j_:W + 3 + j_],
                                              botrows[:, br, 1 - i_:2 - i_, W - 1 - j_:W - j_])
                    nc.sync.dma_start(
                        out=y[g, cs, H + 2:H + 4, :].rearrange("c a b -> c (a b)"),
                        in_=sb_f)

                emitters = dict(mid=emit_mid, top=emit_top, bot=emit_bot)

                def load(f, pending):
                    tile = tpool.tile([P, H, W], f32,
                                      name=f"tile_{c0}_{f}", tag="tile")
                    nc.sync.dma_start(
                        out=tile.rearrange("p a b -> p (a b)"),
                        in_=x[f, cs].rearrange("c a b -> c (a b)"))
                    V.tensor_copy(toprows[:, f], tile[:, 0:2, :])
                    V.tensor_copy(botrows[:, f], tile[:, H - 2:H, :])
                    V.tensor_copy(colL[:, f], tile[:, :, 0:2])
                    V.tensor_copy(colR[:, f], tile[:, :, W - 2:W])
                    mid = mpool.tile([P, H, OH], f32,
                                     name=f"mid_{c0}_{f}", tag="mid")
                    V.tensor_copy(mid[:, :, 2:2 + W], tile[:])
                    mids[f] = mid
                    loaded.add(f)
                    for g in _ORDER:
                        for piece, deps in _piece_deps(g).items():
                            if (g, piece) in emitted or not deps <= loaded:
                                continue
                            emitted.add((g, piece))
                            pending.append((emitters[piece], piece, g))

                return load, emitted

            # Flat schedule over both 128-channel chunks: pieces are emitted
            # through a paced queue so chunk-0's tail stores interleave with
            # chunk-1's first loads instead of starving the DMA engines on
            # back-to-back descriptor generation of tiny strip stores. Mid
            # stores drain before strips: a 6.2us mid transfer hides the
            # HWDGE generation time of the following strips.
            chunks = [make_chunk(c0) for c0 in range(0, C, PCHUNK)]
            pending = []
            CAP = 4
            for s in range(len(chunks) * _FACES):
                chunks[s // _FACES][0](_ORDER[s % _FACES], pending)
                pending.sort(key=lambda e: e[1] != "mid")
                for _ in range(min(CAP, len(pending))):
                    fn, piece, g = pending.pop(0)
                    fn(g)
            tail_mids = [e for e in pending if e[1] == "mid"]
            tail_rest = [e for e in pending if e[1] != "mid"]
            for fn, piece, g in tail_mids[:1] + tail_rest + tail_mids[1:]:
                fn(g)
            for _, emitted in chunks:
                assert len(emitted) == 3 * _FACES

    # walrus's CoreV2 codegen accepts a single sync-wait slot per instruction
    # ("Too many sync wait commands" in setupSyncWait), but the Tile framework
    # legitimately gives some instructions several waits: DMAs get a compute
    # wait plus their DMAHW-lane slot-reuse wait (which keeps each lane's
    # completion updates ordered — required for the shared counting-sem
    # protocol on hardware, where DMA engines finish out of order), and the
    # kernel-tail Drains join multiple sems. Splitting is sound on in-order
    # sequencers: excess waits move to no-op InstEventSemaphore carriers
    # inserted immediately before the instruction on the same engine, so the
    # original instruction still cannot issue before every wait is satisfied.
    # DMAs keep the wait on the DMAHW lane they update so each lane's +16
    # completion updates stay ordered with its predecessor's.
    import concourse.mybir as mybir
    for blk in nc.m.functions[0].blocks:
        out_insts = []
        for inst in blk.instructions:
            si = inst.sync_info
            if si is None or len(si.on_wait) <= 1:
                out_insts.append(inst)
                continue
            waits = list(si.on_wait)
            own_lanes = {u.ant_name for u in si.on_update}
            keep_idx = next((i for i, w in enumerate(waits)
                             if w.ant_name in own_lanes), len(waits) - 1)
            keep = waits.pop(keep_idx)
            for w in waits:
                carrier = mybir.InstEventSemaphore(
                    name=f"I-{nc.next_id()}", engine=inst.engine)
                carrier.sync_info = mybir.SyncInfo(on_wait=[w], on_update=[])
                nc.inst_map[carrier.name] = carrier
                out_insts.append(carrier)
            si.on_wait = [keep]
            inst.sync_info = si
            out_insts.append(inst)
        blk.instructions = out_insts

    for blk in nc.m.functions[0].blocks:
        for inst in blk.instructions:
            si = inst.sync_info
            assert si is None or len(si.on_wait) <= 1, inst.concise()

    # The first load has no waits, but sits after the 5-engine startup
    # barrier (~700ns) purely by block position. Hoist it into the preamble
    # block directly after SP's entry Drain: the Drain's sem/DMA-state reset
    # still precedes it on the in-order SP stream, and its completion-sem
    # update lands only after the ~6us transfer — microseconds after every
    # engine's init Drain — so the barrier provides it no ordering it needs.
    # This starts the first transfer ~760ns earlier.
    b0 = nc.m.functions[0].blocks[0]
    b1 = nc.m.functions[0].blocks[1]
    first = b1.instructions[0]
    assert isinstance(first, mybir.InstDMACopy), first.concise()
    assert first.engine == mybir.EngineType.SP, first.concise()
    assert first.sync_info is None or not first.sync_info.on_wait, first.concise()
    sp_drain = [i for i, x in enumerate(b0.instructions)
                if x.engine == mybir.EngineType.SP and isinstance(x, mybir.InstDrain)]
    assert len(sp_drain) == 1, sp_drain
    b1.instructions = b1.instructions[1:]
    b0.instructions = (b0.instructions[:sp_drain[0] + 1] + [first]
                       + b0.instructions[sp_drain[0] + 1:])

    # Further: SP's five preamble RegisterMoves (zero + broadcast regs) are
    # SP-local and nothing in the SP stream reads them; running them during
    # the hoisted DMA's descriptor-gen/DGE latency instead of before it
    # starts the first transfer another 250ns earlier. Reorder SP's
    # sub-sequence to [Drain, DMA, RegisterMoves..., barrier, branch] —
    # other engines' streams are untouched (each engine executes only its
    # own filtered subsequence).
    sp_idx = [i for i, x in enumerate(b0.instructions)
              if x.engine == mybir.EngineType.SP]
    sp_seq = [b0.instructions[i] for i in sp_idx]
    moves = [x for x in sp_seq if isinstance(x, mybir.InstRegisterMove)]
    drain = [x for x in sp_seq if isinstance(x, mybir.InstDrain)]
    dma = [x for x in sp_seq if isinstance(x, mybir.InstDMACopy)]
    rest = [x for x in sp_seq
            if not isinstance(x, (mybir.InstRegisterMove, mybir.InstDrain,
                                  mybir.InstDMACopy))]
    assert len(drain) == 1 and len(dma) == 1 and len(moves) == 5, (
        len(drain), len(dma), len(moves))
    new_sp = drain + dma + moves + rest
    for i, inst in zip(sp_idx, new_sp):
        b0.instructions[i] = inst

    nc.finalize()
    return nc


_NC_CACHE = {}


def _get_nc():
    if "nc" not in _NC_CACHE:
        _NC_CACHE["nc"] = _build_nc()
    return _NC_CACHE["nc"]


def _run(data, **kwargs):
    from concourse import bass_utils

    data = np.ascontiguousarray(np.asarray(data, dtype=np.float32))
    n_cores = 8
    group = data.shape[0] // n_cores
    assert group == _FACES
    nc = _get_nc()
    in_maps = [{"data": data[g * group:(g + 1) * group]} for g in range(n_cores)]
    return bass_utils.run_bass_kernel_spmd(
        nc, in_maps, core_ids=list(range(n_cores)), **kwargs)


def kernel(data, p):
    assert int(p) == _PAD
    res = _run(data)
    return np.concatenate([r["out"] for r in res.results], axis=0)
